# revision 19
# baseline (speedup 1.0000x reference)
"""Trainium2 Bass kernel for nn_Kenn_across (gnn_message_passing).

Pipeline on 8 NeuronCores (SPMD), per core:
  MLP     across = relu(featT @ W1 + b1) @ W2 + b2 on RPAD=25088 rows,
          features host-pretransposed to bf16 [DC,128,RPAD] (no PE transposes).
  AG      chunked AllGather of across slices into the replicated pre table.
          Only chunks 0..2 are AllGathered: the last chunk holds exactly the
          across rows that NO xy/yz gather references (~26% of rows), so it
          needs no collective and the whole gather+KENN pipeline for the
          first chunks hides under the last chunk's MLP window.
  gather  per-column [128,1] indirect DMAs for u_xy / u_yz, issued in
          dependency order (within-table columns at t=0, chunk-c columns
          right after AG_c). u_xz needs NO gather: each across row's winning
          triple has index_xz == that row, so u_xz == own across value.
  KENN    3 clause-enhancement layers + masked select + softmax, split into
          group A (chunks 0-2 columns, emitted mid-MLP so it overlaps the
          last-chunk matmuls) and group B (last-chunk columns, after MLP).

Host-side prep (integer bookkeeping + feature permute/bf16):
  - winner permutation per across row (last-write-wins scatter semantics)
  - referenced-set computation; unreferenced rows dealt to the last chunk
  - global column-synchronized class sort: rows sorted by (xy-dep, yz-dep)
    within each chunk and dealt across cores column-by-column, so gather
    columns have identical dependency classes on every core; mask-0 slots
    sort first and their columns are skipped entirely
  - gather indices rewritten to the device table layout
"""
import hashlib
import numpy as np
import ml_dtypes

import concourse.bass as bass
import concourse.mybir as mybir
import concourse.tile as tile
from concourse import bacc
from concourse.bass_utils import run_bass_kernel_spmd

# problem constants (spec: nn_Kenn_across_29661044146692)
N_CORES = 8
NW, NA, M = 100000, 200000, 262144
D, H = 1024, 1024
RPC = NA // N_CORES              # 25000 across rows per core
PS = 196                         # columns (slots per partition)
RPAD = 128 * PS                  # 25088 padded slots per core
PADS = N_CORES * RPAD - NA       # 704 global pad slots (in the last chunk)
DC = D // 128
HC = H // 128
IT = 512                         # MLP tile rows; 49 * 512 == RPAD
NTILES = RPAD // IT
N_LAYERS = 3
NCH = 4                          # chunks; last one is not AllGathered
A_READ_TILE = 38                 # emit group-A u_xz read after this MLP tile
A_LAYER_TILES = (44, 46, 48)     # emit group-A KENN layer l after these tiles

f32 = mybir.dt.float32
bf16 = mybir.dt.bfloat16
i32 = mybir.dt.int32
i8 = mybir.dt.int8
AF = mybir.ActivationFunctionType
ALU = mybir.AluOpType

_CACHE = {}


def _build(gather_plan, sc_cols):
    """gather_plan: sorted list of (ub_rows, name, col); the indirect gather
    for that column reads pre[0:ub_rows]. sc_cols: columns per chunk."""
    s_cum = [0]
    for c in sc_cols:
        s_cum.append(s_cum[-1] + c * 128)
    assert s_cum[-1] == RPAD
    nt3 = NW + N_CORES * s_cum[NCH - 1]       # pre rows (last chunk excluded)
    ag_tiles = {s_cum[c + 1] // IT - 1: c for c in range(NCH - 1)}
    cut_ubs = [NW] + [NW + N_CORES * s_cum[c + 1] for c in range(NCH - 1)]
    cuts = [sum(1 for ub, _, _ in gather_plan if ub <= u) for u in cut_ubs]
    c012 = s_cum[NCH - 1] // 128              # group-A columns [0, c012)

    nc = bacc.Bacc("TRN2", target_bir_lowering=False, debug=False,
                   num_devices=N_CORES)

    featT = nc.dram_tensor("featT", [DC, 128, RPAD], bf16, kind="ExternalInput")
    w1b = nc.dram_tensor("w1b", [DC, 128, H], bf16, kind="ExternalInput")
    w2b = nc.dram_tensor("w2b", [HC, 128, 3], bf16, kind="ExternalInput")
    b1 = nc.dram_tensor("b1", [H], f32, kind="ExternalInput")
    b2 = nc.dram_tensor("b2", [3, 1], f32, kind="ExternalInput")
    win3 = nc.dram_tensor("win3", [NW, 3], f32, kind="ExternalInput")
    cw = nc.dram_tensor("cw", [128, N_LAYERS * 3], f32, kind="ExternalInput")
    isx = nc.dram_tensor("isx", [128, PS], i32, kind="ExternalInput")
    isy = nc.dram_tensor("isy", [128, PS], i32, kind="ExternalInput")
    pmask = nc.dram_tensor("pmask", [128, PS], i8, kind="ExternalInput")

    out_o = nc.dram_tensor("out_o", [128, PS * 3], f32, kind="ExternalOutput")
    sm_o = nc.dram_tensor("sm_o", [128, PS * 3], f32, kind="ExternalOutput")

    with tile.TileContext(nc) as tc:
        with tc.tile_pool(name="const", bufs=1) as cp, \
             tc.tile_pool(name="work", bufs=2) as wp, \
             tc.tile_pool(name="ps1", bufs=2, space="PSUM") as pp1, \
             tc.tile_pool(name="ps2", bufs=2, space="PSUM") as pp2, \
             tc.tile_pool(name="dram", bufs=1, space="DRAM") as dp:

            # ---------- constants ----------
            # only w1 (needed by the first matmul) rides the sync queue, so
            # the first at-tile load is not delayed; everything else goes via
            # the scalar engine's DMA queue (Act SEQ is free early on)
            w1sb = cp.tile([128, DC, H], bf16)
            nc.sync.dma_start(w1sb[:], w1b[:].rearrange("d p h -> p d h"))
            w2sb = cp.tile([128, HC, 3], bf16)
            nc.scalar.dma_start(w2sb[:], w2b[:].rearrange("h p c -> p h c"))
            b1sb = cp.tile([128, HC], f32)
            nc.scalar.dma_start(b1sb[:], b1[:].rearrange("(hc p) -> p hc", p=128))
            b2sb = cp.tile([3, 1], f32)
            nc.scalar.dma_start(b2sb[:], b2[:])
            cwsb = cp.tile([128, 1, N_LAYERS * 3], f32)
            nc.scalar.dma_start(cwsb[:], cw[:])
            isxsb = cp.tile([128, PS], i32)
            nc.scalar.dma_start(isxsb[:], isx[:])
            isysb = cp.tile([128, PS], i32)
            nc.scalar.dma_start(isysb[:], isy[:])
            msb = cp.tile([128, PS, 1], i8)
            nc.scalar.dma_start(msb[:], pmask[:])

            pre = dp.tile([nt3, 3], f32)
            acc = dp.tile([RPAD, 3], f32)
            agt = []
            for c in range(NCH - 1):
                agt_c = dp.tile([N_CORES * (s_cum[c + 1] - s_cum[c]), 3], f32,
                                addr_space="Shared", name=f"agt{c}")
                agt.append(agt_c)
            nc.scalar.dma_start(pre[0:NW, :], win3[:])

            uxy = wp.tile([128, PS, 3], f32, tag="uxy", bufs=1)
            uyz = wp.tile([128, PS, 3], f32, tag="uyz", bufs=1)
            uxz = wp.tile([128, PS, 3], f32, tag="uxz", bufs=1)
            acr = wp.tile([128, PS, 3], f32, tag="acr", bufs=1)
            nc.vector.memset(uxy[:], 0.0)
            nc.vector.memset(uyz[:], 0.0)

            def issue_gathers(lo, hi):
                for ub, name, s in gather_plan[lo:hi]:
                    t_ = uxy if name == "xy" else uyz
                    sb_ = isxsb if name == "xy" else isysb
                    nc.gpsimd.indirect_dma_start(
                        out=t_[:, s, :], out_offset=None, in_=pre[0:ub, :],
                        in_offset=bass.IndirectOffsetOnAxis(
                            ap=sb_[:, s:s + 1], axis=0))

            u = {"xy": uxy, "yz": uyz, "xz": uxz}

            def kenn_layer(lo, hi, l, g):
                """One KENN layer on columns [lo,hi). xy/yz updates run on
                the Pool engine (free by now), xz on DVE."""
                w = hi - lo
                exy = wp.tile([128, w, 3], f32, tag=f"exy{g}", name=f"exy{g}")
                eyz = wp.tile([128, w, 3], f32, tag=f"eyz{g}", name=f"eyz{g}")
                exz = wp.tile([128, w, 3], f32, tag=f"exz{g}", name=f"exz{g}")
                nc.scalar.activation(exy[:], u["xy"][:, lo:hi, :], AF.Exp,
                                     scale=-1.0)
                nc.scalar.activation(eyz[:], u["yz"][:, lo:hi, :], AF.Exp,
                                     scale=-1.0)
                nc.scalar.activation(exz[:], u["xz"][:, lo:hi, :], AF.Exp,
                                     scale=1.0)
                ssum = wp.tile([128, w, 3], f32, tag=f"ssum{g}", name=f"ssum{g}")
                nc.vector.tensor_tensor(ssum[:], exy[:], eyz[:], op=ALU.add)
                nc.vector.tensor_tensor(ssum[:], ssum[:], exz[:], op=ALU.add)
                nc.vector.reciprocal(ssum[:], ssum[:])
                rw = wp.tile([128, w, 3], f32, tag=f"rw{g}", name=f"rw{g}")
                cwb = cwsb[:, :, l * 3:(l + 1) * 3].to_broadcast([128, w, 3])
                nc.vector.tensor_tensor(rw[:], ssum[:], cwb, op=ALU.mult)
                for name, op in (("xy", ALU.subtract), ("yz", ALU.subtract)):
                    e = {"xy": exy, "yz": eyz}[name]
                    nc.gpsimd.tensor_tensor(e[:], e[:], rw[:], op=ALU.mult)
                    nc.gpsimd.tensor_tensor(u[name][:, lo:hi, :],
                                            u[name][:, lo:hi, :], e[:], op=op)
                nc.vector.tensor_tensor(exz[:], exz[:], rw[:], op=ALU.mult)
                nc.vector.tensor_tensor(u["xz"][:, lo:hi, :],
                                        u["xz"][:, lo:hi, :], exz[:], op=ALU.add)

            def select_softmax(lo, hi, g, out_engine):
                """Masked select vs across + softmax + output DMAs."""
                w = hi - lo
                nc.vector.copy_predicated(
                    acr[:, lo:hi, :],
                    msb[:, lo:hi, :].to_broadcast([128, w, 3]),
                    u["xz"][:, lo:hi, :])
                a3 = acr[:, lo:hi, :]
                mx = wp.tile([128, w], f32, tag=f"mx{g}", name=f"mx{g}")
                nc.vector.tensor_tensor(mx[:], a3[:, :, 0], a3[:, :, 1],
                                        op=ALU.max)
                nc.vector.tensor_tensor(mx[:], mx[:], a3[:, :, 2], op=ALU.max)
                e3 = wp.tile([128, w, 3], f32, tag=f"e3{g}", name=f"e3{g}")
                for cc in range(3):
                    nc.vector.tensor_tensor(e3[:, :, cc], a3[:, :, cc], mx[:],
                                            op=ALU.subtract)
                nc.scalar.activation(e3[:], e3[:], AF.Exp, scale=1.0)
                ssm = wp.tile([128, w], f32, tag=f"ssm{g}", name=f"ssm{g}")
                nc.vector.tensor_tensor(ssm[:], e3[:, :, 0], e3[:, :, 1],
                                        op=ALU.add)
                nc.vector.tensor_tensor(ssm[:], ssm[:], e3[:, :, 2], op=ALU.add)
                nc.vector.reciprocal(ssm[:], ssm[:])
                sm = wp.tile([128, w, 3], f32, tag=f"sm{g}", name=f"sm{g}")
                for cc in range(3):
                    nc.vector.tensor_tensor(sm[:, :, cc], e3[:, :, cc], ssm[:],
                                            op=ALU.mult)
                out_engine.dma_start(
                    out_o[:, lo * 3:hi * 3],
                    acr[:, lo:hi, :].rearrange("p s c -> p (s c)"))
                out_engine.dma_start(
                    sm_o[:, lo * 3:hi * 3],
                    sm[:].rearrange("p s c -> p (s c)"))

            issue_gathers(0, cuts[0])          # within-table columns

            # ---------- MLP ----------
            for t in range(NTILES):
                r0 = t * IT
                at = wp.tile([128, DC, IT], bf16, tag="at", bufs=3)
                nc.sync.dma_start(
                    at[:], featT[:, :, r0:r0 + IT].rearrange("d p r -> p d r"))
                c1t = wp.tile([128, HC, IT], bf16, tag="c1t")
                for hc in range(HC):
                    p1 = pp1.tile([128, IT], f32, tag="p1")
                    for dc in range(DC):
                        nc.tensor.matmul(p1[:],
                                         lhsT=w1sb[:, dc, hc * 128:(hc + 1) * 128],
                                         rhs=at[:, dc, :],
                                         start=(dc == 0), stop=(dc == DC - 1))
                    nc.scalar.activation(c1t[:, hc, :], p1[:], AF.Relu,
                                         bias=b1sb[:, hc:hc + 1], scale=1.0)
                p2 = pp2.tile([3, IT], f32, tag="p2")
                for hc in range(HC):
                    nc.tensor.matmul(p2[:], lhsT=w2sb[:, hc, :],
                                     rhs=c1t[:, hc, :],
                                     start=(hc == 0), stop=(hc == HC - 1))
                acc_sb = wp.tile([3, IT], f32, tag="acc", bufs=3)
                nc.scalar.activation(acc_sb[:], p2[:], AF.Identity,
                                     bias=b2sb[:, 0:1], scale=1.0)
                nc.sync.dma_start(
                    acc[r0:r0 + IT, :].rearrange("r c -> c r"), acc_sb[:])

                if t in ag_tiles:
                    c = ag_tiles[t]
                    nc.gpsimd.collective_compute(
                        "AllGather", ALU.bypass,
                        replica_groups=[list(range(N_CORES))],
                        ins=[acc[s_cum[c]:s_cum[c + 1], :]],
                        outs=[agt[c][:]])
                    # pre-copy rides the Pool DMA queue so its wait on the
                    # collective cannot head-of-line-block the at-tile loads
                    # (everything later on Pool SEQ already depends on it)
                    nc.gpsimd.dma_start(
                        pre[NW + N_CORES * s_cum[c]:
                            NW + N_CORES * s_cum[c + 1], :], agt[c][:])
                    issue_gathers(cuts[c], cuts[c + 1])

                if t == A_READ_TILE:
                    # group A u_xz/acr: own across values (chunks 0-2 done);
                    # two DMAs to stay under the 16384-descriptor limit
                    ch = c012 // 2
                    nc.gpsimd.dma_start(
                        uxz[:, 0:ch, :],
                        acc[0:ch * 128, :].rearrange("(s p) c -> p s c", p=128))
                    nc.gpsimd.dma_start(
                        uxz[:, ch:c012, :],
                        acc[ch * 128:s_cum[NCH - 1], :].rearrange(
                            "(s p) c -> p s c", p=128))
                    nc.vector.tensor_copy(acr[:, 0:c012, :], uxz[:, 0:c012, :])

                if t in A_LAYER_TILES:
                    # staged so each layer's exps are ready when the Act SEQ
                    # reaches them (no wait-queue stall of the MLP acts)
                    kenn_layer(0, c012, A_LAYER_TILES.index(t), "a")

            # ---------- group A final ----------
            select_softmax(0, c012, "a", nc.gpsimd)

            # ---------- group B: last-chunk columns ----------
            nc.sync.dma_start(
                uxz[:, c012:PS, :],
                acc[s_cum[NCH - 1]:RPAD, :].rearrange("(s p) c -> p s c", p=128))
            nc.vector.tensor_copy(acr[:, c012:PS, :], uxz[:, c012:PS, :])
            for l in range(N_LAYERS):
                kenn_layer(c012, PS, l, "b")
            select_softmax(c012, PS, "b", nc.sync)

    nc.compile()
    return nc


def kernel(features, within_pre, index_xy, index_yz, index_xz,
           W1, b1, W2, b2, clause_weights):
    features = np.asarray(features)
    within_pre = np.asarray(within_pre)
    index_xy = np.asarray(index_xy, np.int64)
    index_yz = np.asarray(index_yz, np.int64)
    index_xz = np.asarray(index_xz, np.int64)

    NT0 = NW + NA
    # winner per across row (numpy last-write-wins scatter semantics)
    perm_full = np.full(NT0, -1, np.int64)
    perm_full[index_xz] = np.arange(M)
    perm_a = perm_full[NW:]
    mask_a = perm_a >= 0
    pidx_a = np.where(mask_a, perm_a, 0)
    ixy_w = index_xy[pidx_a]          # per global across row j: winner's xy idx
    iyz_w = index_yz[pidx_a]

    # referenced across rows (by any mask-1 slot's xy/yz)
    referenced = np.zeros(NA, bool)
    for idx_w in (ixy_w, iyz_w):
        tgt = idx_w[mask_a]
        tgt = tgt[tgt >= NW] - NW
        referenced[tgt] = True
    unref_ids = np.nonzero(~referenced)[0]
    ref_ids = np.nonzero(referenced)[0]

    # chunk geometry: last chunk = unreferenced rows only (no AllGather)
    cols3 = min(((len(unref_ids) + PADS) // 1024) // 4 * 4, 64)
    cols3 = max(cols3, 0)
    cap3_rows = cols3 * 1024 - PADS
    rem = PS - cols3
    c0 = (rem // 3) // 4 * 4
    c1 = (rem // 3) // 4 * 4
    sc_cols = (c0, c1, rem - c0 - c1, cols3)
    s_cum = np.concatenate([[0], np.cumsum([c * 128 for c in sc_cols])])
    cap = np.asarray([N_CORES * 128 * c for c in sc_cols], np.int64)

    # fixed chunk assignment (pre-sort): referenced rows (plus unreferenced
    # spill) deal sequentially into chunks 0..2; the rest fill chunk 3
    chunk3_rows = unref_ids[:cap3_rows]
    relike = np.sort(np.concatenate([ref_ids, unref_ids[cap3_rows:]]))
    chunk_of_row = np.empty(NA, np.int64)
    chunk_of_row[chunk3_rows] = NCH - 1
    b0 = int(cap[0])
    b1_ = b0 + int(cap[1])
    chunk_of_row[relike[:b0]] = 0
    chunk_of_row[relike[b0:b1_]] = 1
    chunk_of_row[relike[b1_:]] = 2
    row_lists = [relike[:b0], relike[b0:b1_], relike[b1_:], chunk3_rows]

    # dep class per global across row for each name:
    #   -2 no need (mask 0), -1 within-table, 0..2 referenced AG chunk
    def dep_class(idx_w):
        is_w = idx_w < NW
        cls = np.where(is_w, -1, chunk_of_row[np.maximum(idx_w - NW, 0)])
        return np.where(mask_a, cls, -2).astype(np.int64)

    cx = dep_class(ixy_w)
    cy = dep_class(iyz_w)
    assert cx.max() <= NCH - 2 and cy.max() <= NCH - 2

    # global within-chunk sort by (cx, cy), column-synchronized deal
    key_all = (cx + 2) * (NCH + 2) + (cy + 2)
    row_at_slot_g = np.full((N_CORES, RPAD), -1, np.int64)  # global row ids
    pos_of_global = np.full(NA, -1, np.int64)               # device table row
    plan_cols = {"xy": np.full(PS, -2, np.int64),
                 "yz": np.full(PS, -2, np.int64)}
    for c in range(NCH):
        rows_c = row_lists[c]
        order = rows_c[np.argsort(key_all[rows_c], kind="stable")]
        L = np.concatenate([order, np.full(int(cap[c]) - len(rows_c), -1,
                                           np.int64)])
        idx = np.arange(len(L))
        col_g = idx // (N_CORES * 128)
        core = (idx % (N_CORES * 128)) // 128
        p = idx % 128
        s = s_cum[c] // 128 + col_g
        l_slot = s * 128 + p
        row_at_slot_g[core, l_slot] = L
        real = L >= 0
        pos_of_global[L[real]] = (NW + N_CORES * s_cum[c]
                                  + core[real] * (s_cum[c + 1] - s_cum[c])
                                  + col_g[real] * 128 + p[real])
        for name, cls in (("xy", cx), ("yz", cy)):
            cls_L = np.where(real, cls[np.where(real, L, 0)], -2)
            for cg in range(int(cap[c]) // (N_CORES * 128)):
                span = cls_L[cg * N_CORES * 128:(cg + 1) * N_CORES * 128]
                plan_cols[name][s_cum[c] // 128 + cg] = span.max()

    # rewritten per-core per-slot gather indices + mask
    isx_np = np.zeros((N_CORES, 128, PS), np.int32)
    isy_np = np.zeros((N_CORES, 128, PS), np.int32)
    msk_np = np.zeros((N_CORES, 128, PS), np.int8)
    for k in range(N_CORES):
        ras = row_at_slot_g[k]
        valid = ras >= 0
        g = np.where(valid, ras, 0)
        m_slot = np.where(valid, mask_a[g], False)

        def rewrite(idx_w):
            i0 = idx_w[g]
            a = np.maximum(i0 - NW, 0)
            out = np.where(i0 < NW, i0, pos_of_global[a])
            return np.where(m_slot, out, 0).astype(np.int32)

        isx_np[k] = rewrite(ixy_w).reshape(PS, 128).T
        isy_np[k] = rewrite(iyz_w).reshape(PS, 128).T
        msk_np[k] = m_slot.astype(np.int8).reshape(PS, 128).T
    assert isx_np.max() < NW + N_CORES * int(s_cum[NCH - 1])
    assert isy_np.max() < NW + N_CORES * int(s_cum[NCH - 1])

    gather_plan = []
    for name in ("xy", "yz"):
        for s in range(PS):
            d = int(plan_cols[name][s])
            if d == -2:
                continue                         # no slot needs this column
            ub = NW if d == -1 else NW + N_CORES * int(s_cum[d + 1])
            gather_plan.append((ub, name, s))
    gather_plan.sort(key=lambda e: e[0])

    plan_key = hashlib.sha256(
        (repr(gather_plan) + repr(sc_cols)).encode()).hexdigest()
    if _CACHE.get("plan_key") != plan_key:
        _CACHE["nc"] = _build(gather_plan, sc_cols)
        _CACHE["plan_key"] = plan_key
    nc = _CACHE["nc"]

    # ---------- numeric inputs ----------
    w1bf = np.ascontiguousarray(
        np.asarray(W1, np.float32).reshape(DC, 128, H)).astype(ml_dtypes.bfloat16)
    w2bf = np.ascontiguousarray(
        np.asarray(W2, np.float32).reshape(HC, 128, 3)).astype(ml_dtypes.bfloat16)
    b1f = np.asarray(b1, np.float32)
    b2r = np.asarray(b2, np.float32).reshape(3, 1)
    win3 = np.ascontiguousarray(within_pre[:, :3], np.float32)
    cwb = np.broadcast_to(
        np.asarray(clause_weights, np.float32).reshape(1, N_LAYERS * 3),
        (128, N_LAYERS * 3)).copy()

    in_maps = []
    for k in range(N_CORES):
        ras = row_at_slot_g[k]
        src = np.where(ras >= 0, ras, 0)
        fp = features[src].astype(ml_dtypes.bfloat16)
        fp[ras < 0] = 0
        featT_k = np.ascontiguousarray(fp.T.reshape(DC, 128, RPAD))
        in_maps.append({
            "featT": featT_k,
            "w1b": w1bf, "w2b": w2bf, "b1": b1f, "b2": b2r,
            "win3": win3, "cw": cwb,
            "isx": isx_np[k], "isy": isy_np[k], "pmask": msk_np[k],
        })

    res = run_bass_kernel_spmd(nc, in_maps, core_ids=list(range(N_CORES)))
    _CACHE["last_results"] = res

    out = np.empty((NA, 3), np.float32)
    smx = np.empty((NA, 3), np.float32)
    for k in range(N_CORES):
        raw_o = res.results[k]["out_o"].reshape(128, PS, 3)
        raw_s = res.results[k]["sm_o"].reshape(128, PS, 3)
        o_slot = raw_o.transpose(1, 0, 2).reshape(RPAD, 3)
        s_slot = raw_s.transpose(1, 0, 2).reshape(RPAD, 3)
        ras = row_at_slot_g[k]
        valid = ras >= 0
        out[ras[valid]] = o_slot[valid]
        smx[ras[valid]] = s_slot[valid]
    return out, smx
